# revision 9
# baseline (speedup 1.0000x reference)
"""MetaGraphSAGE Trainium2 kernel (8 NeuronCores, Bass/Tile), bf16 edge path.

Per metagraph (3 independent graphs):
    h  = ELU(mean_agg(x) @ W1l + x @ W1r + b1)
    o  = mean_agg(h) @ W2l + h @ W2r + b2
    out = log_softmax(o, axis=1)

Sharding: nodes padded to 8*6272=50176; core c owns dst nodes
[c*6272,(c+1)*6272). Edges are partitioned by dst owner on the host and
sorted by 128-wide dst block. On device, messages are gathered with
dma_gather (256B bf16 rows) and segment-summed via bf16 one-hot matmuls
accumulating in fp32 PSUM as aggT [128feat x 128dst] for BOTH layers.
Layer 2 gathers bf16 h rows from an in-kernel AllGather output (h rows
are produced per block by a PE transpose of the ELU output). int16
gather indices force a lo/hi split of each block-group's edge list at
src=32768. Dense matmuls and ELU batch over GRP=4 blocks; log_softmax
batches over each group to limit activation-table thrash.
"""

import sys

sys.path.insert(0, "/opt/trn_rl_repo")

import ml_dtypes
import numpy as np

META, N, E, F, H, D = 3, 50000, 640000, 128, 128, 64
NCORES = 8
NSH = 6272            # padded nodes per core (6272*8 = 50176 >= N)
NPAD = NSH * NCORES
NBLK = NSH // 128     # 49 blocks of 128 dst nodes per core
GRP = 4               # dst blocks per psum bank-pair / dense batch
WIN = 5               # max 128-edge chunks per dma_gather call (HW cap ~1024 idx)
SPLIT = 32768         # int16 gather index limit
DMA_SCRATCH = 49152   # per-partition SWDGE descriptor carveout bytes
NQUEUE = 4            # SWDGE queues for gather round-robin (ucode max)

BF16 = ml_dtypes.bfloat16


def _ceil(a, b):
    return (a + b - 1) // b


def _ceil_arr(a, b):
    return (a + b - 1) // b


def _prep_host(meta_x, meta_edge_index):
    """Partition+sort edges per (core, graph); build all per-core arrays."""
    meta_x = np.asarray(meta_x, dtype=np.float32)
    ei = np.asarray(meta_edge_index, dtype=np.int64)

    counts_lo = np.zeros((NCORES, META, NBLK), dtype=np.int64)
    counts_hi = np.zeros((NCORES, META, NBLK), dtype=np.int64)
    edata = {}  # (c,g,blk,seg) -> (src_idx_array, d128_array)
    inv_all = np.zeros((NCORES, META, NSH), dtype=np.float32)
    for g in range(META):
        src = ei[g, 0]
        dst = ei[g, 1]
        core = dst // NSH
        for c in range(NCORES):
            sel = core == c
            s = src[sel]
            dl = dst[sel] - c * NSH
            cnt = np.bincount(dl, minlength=NSH).astype(np.float32)
            inv_all[c, g] = 1.0 / np.maximum(cnt, 1.0)
            blk = dl // 128
            d128 = dl % 128
            hi = (s >= SPLIT).astype(np.int64)
            order = np.lexsort((hi, blk))
            s, d128, blk, hi = s[order], d128[order], blk[order], hi[order]
            for b in range(NBLK):
                mb = blk == b
                sb, db, hb = s[mb], d128[mb], hi[mb]
                lo_n = int((hb == 0).sum())
                counts_lo[c, g, b] = lo_n
                counts_hi[c, g, b] = len(sb) - lo_n
                edata[(c, g, b, 0)] = (sb[:lo_n], db[:lo_n])
                edata[(c, g, b, 1)] = (sb[lo_n:] - SPLIT, db[lo_n:])

    # padded chunk counts per (g, blk, seg): max over cores, in 128-chunks
    pl = np.maximum(_ceil_arr(counts_lo.max(axis=0), 128), 1)  # [META,NBLK]
    ph = np.maximum(_ceil_arr(counts_hi.max(axis=0), 128), 1)

    # stream layout per graph: groups of GRP blocks, [lo segs | hi segs]
    stream = [[] for _ in range(META)]   # per g: list of (blk, seg)
    groups = []  # (g, blk0, nblk, lo_chunk0, lo_nchunk, hi_chunk0, hi_nchunk)
    for g in range(META):
        for b0 in range(0, NBLK, GRP):
            nb = min(GRP, NBLK - b0)
            lo0 = len(stream[g])
            for b in range(b0, b0 + nb):
                stream[g] += [(b, 0)] * int(pl[g, b])
            hi0 = len(stream[g])
            for b in range(b0, b0 + nb):
                stream[g] += [(b, 1)] * int(ph[g, b])
            groups.append((g, b0, nb, lo0, hi0 - lo0, hi0, len(stream[g]) - hi0))
    totc = [len(stream[g]) for g in range(META)]  # chunks per graph

    # per-core flat idx/dst arrays in stream order (shared by both layers)
    per_core = []
    for c in range(NCORES):
        idx16 = [np.zeros((t * 128,), dtype=np.int16) for t in totc]
        dstf = [np.full((t * 128,), -1.0, dtype=np.float32) for t in totc]
        for g in range(META):
            pos = 0
            cur = None
            for (b, seg) in stream[g]:
                if cur != (b, seg):
                    cur = (b, seg)
                    sarr, darr = edata[(c, g, b, seg)]
                    off = 0
                n = min(128, len(sarr) - off) if len(sarr) > off else 0
                if n > 0:
                    idx16[g][pos : pos + n] = sarr[off : off + n].astype(np.int16)
                    dstf[g][pos : pos + n] = darr[off : off + n].astype(np.float32)
                off += 128
                pos += 128
        idxw = np.concatenate(
            [np.tile(a.reshape(-1, 16).T, (8, 1)) for a in idx16], axis=1
        )  # [128, sum(totc)*8]
        dstw = np.concatenate(
            [a.reshape(-1, 128).T for a in dstf], axis=1
        ).astype(BF16)  # [128, sum(totc)]
        xts = np.zeros((META, 128, NSH), dtype=BF16)
        lo = c * NSH
        hi_n = min(NSH, N - lo)
        for g in range(META):
            if hi_n > 0:
                xts[g, :, :hi_n] = meta_x[g, lo : lo + hi_n].T.astype(BF16)
        invb = np.broadcast_to(
            inv_all[c][:, None, :], (META, 128, NSH)
        ).astype(BF16)  # [META,128,NSH]
        per_core.append(dict(idxw=idxw, dstw=dstw, xts=xts, invb=invb))

    layout = dict(stream=stream, groups=groups, totc=totc, pl=pl, ph=ph)
    return layout, per_core


def _build_program(layout):
    import concourse.bass as bass  # noqa: F401
    import concourse.mybir as mybir
    import concourse.tile as tile
    from concourse import bacc

    fp32 = mybir.dt.float32
    bf16 = mybir.dt.bfloat16
    i16 = mybir.dt.int16
    nc = bacc.Bacc(None, dynamic_dma_scratch_size=DMA_SCRATCH, num_swdge_queues=NQUEUE)
    core_ids = list(range(NCORES))

    totc = layout["totc"]
    totc_all = sum(totc)

    # ---- dram I/O ----
    xbf_in = nc.declare_dram_parameter("xbf", [META, N, F], bf16, isOutput=False)
    idx_in = nc.declare_dram_parameter("idx", [128, totc_all * 8], i16, isOutput=False)
    dst_in = nc.declare_dram_parameter("dstw", [128, totc_all], bf16, isOutput=False)
    xts_in = nc.declare_dram_parameter("xts", [META, 128, NSH], bf16, isOutput=False)
    invb_in = nc.declare_dram_parameter("invb", [META, 128, NSH], bf16, isOutput=False)
    w1l_in = nc.declare_dram_parameter("w1l", [META, F, H], bf16, isOutput=False)
    w1r_in = nc.declare_dram_parameter("w1r", [META, F, H], bf16, isOutput=False)
    b1_in = nc.declare_dram_parameter("b1c", [META, H, 1], fp32, isOutput=False)
    w2l_in = nc.declare_dram_parameter("w2l", [META, H, D], bf16, isOutput=False)
    w2r_in = nc.declare_dram_parameter("w2r", [META, H, D], bf16, isOutput=False)
    b2_in = nc.declare_dram_parameter("b2b", [META, 128, D], fp32, isOutput=False)
    iota_in = nc.declare_dram_parameter("iota", [128, 128], bf16, isOutput=False)
    ident_in = nc.declare_dram_parameter("ident", [128, 128], bf16, isOutput=False)
    out_ext = nc.declare_dram_parameter("out", [META, NSH, D], fp32, isOutput=True)

    hshard = [nc.dram_tensor(f"hshard{g}", [NSH, H], bf16) for g in range(META)]
    hfull = [
        nc.dram_tensor(f"hfull{g}", [NPAD, H], bf16, addr_space="Shared")
        for g in range(META)
    ]

    stream = layout["stream"]
    groups = layout["groups"]
    pl, ph = layout["pl"], layout["ph"]
    goff = [sum(totc[:g]) for g in range(META)]

    with tile.TileContext(nc) as tc:
        with (
            tc.tile_pool(name="const", bufs=1) as cpool,
            tc.tile_pool(name="weights", bufs=1) as wpool,
            tc.tile_pool(name="hT", bufs=1) as hpool,
            tc.tile_pool(name="meta", bufs=1) as mpool,
            tc.tile_pool(name="gath", bufs=6) as gpool,
            tc.tile_pool(name="oneh", bufs=6) as opool,
            tc.tile_pool(name="grp", bufs=2) as qpool,
            tc.tile_pool(name="dense", bufs=3) as dpool,
            tc.tile_pool(name="psA", bufs=1, space="PSUM") as psA,
            tc.tile_pool(name="psD", bufs=1, space="PSUM") as psD,
            tc.tile_pool(name="psO", bufs=2, space="PSUM") as psO,
            tc.tile_pool(name="psX", bufs=2, space="PSUM") as psX,
        ):
            iota_t = cpool.tile([128, 128], bf16, tag="iota")
            nc.sync.dma_start(out=iota_t[:], in_=iota_in[:])
            ident_t = cpool.tile([128, 128], bf16, tag="ident")
            nc.sync.dma_start(out=ident_t[:], in_=ident_in[:])

            # resident per-graph constants + metadata
            w1l_t, w1r_t, w2l_t, w2r_t, b1_t, b2_t = [], [], [], [], [], []
            idx_t, dst_t = [], []
            for g in range(META):
                t = wpool.tile([F, H], bf16, tag=f"w1l{g}", name=f"w1l{g}")
                nc.sync.dma_start(out=t[:], in_=w1l_in[g])
                w1l_t.append(t)
                t = wpool.tile([F, H], bf16, tag=f"w1r{g}", name=f"w1r{g}")
                nc.sync.dma_start(out=t[:], in_=w1r_in[g])
                w1r_t.append(t)
                t = wpool.tile([H, D], bf16, tag=f"w2l{g}", name=f"w2l{g}")
                nc.sync.dma_start(out=t[:], in_=w2l_in[g])
                w2l_t.append(t)
                t = wpool.tile([H, D], bf16, tag=f"w2r{g}", name=f"w2r{g}")
                nc.sync.dma_start(out=t[:], in_=w2r_in[g])
                w2r_t.append(t)
                t = wpool.tile([H, 1], fp32, tag=f"b1{g}", name=f"b1{g}")
                nc.sync.dma_start(out=t[:], in_=b1_in[g])
                b1_t.append(t)
                t = wpool.tile([128, D], fp32, tag=f"b2{g}", name=f"b2{g}")
                nc.sync.dma_start(out=t[:], in_=b2_in[g])
                b2_t.append(t)
                t = mpool.tile([128, totc[g] * 8], i16, tag=f"idx{g}", name=f"idx{g}")
                nc.sync.dma_start(
                    out=t[:], in_=idx_in[:, goff[g] * 8 : (goff[g] + totc[g]) * 8]
                )
                idx_t.append(t)
                t = mpool.tile([128, totc[g]], bf16, tag=f"dst{g}", name=f"dst{g}")
                nc.sync.dma_start(
                    out=t[:], in_=dst_in[:, goff[g] : goff[g] + totc[g]]
                )
                dst_t.append(t)

            hT = [
                hpool.tile([H, NSH], bf16, tag=f"hTg{g}", name=f"hTg{g}")
                for g in range(META)
            ]

            qrr = [0]

            def edge_phase(g, layer):
                """Gather + one-hot matmul accumulate for all groups of graph g.

                Both layers produce psum aggT [128feat x 128dst] per (block,
                seg) via lhsT=gathered bf16 rows, rhs=bf16 one-hot. Yields
                (b0, nb, psum_slice) per group once its psums are complete.
                """
                pool = psA if layer == 1 else psD
                for (gg, b0, nb, lo0, lon, hi0, hin) in groups:
                    if gg != g:
                        continue
                    # one psum region [128 cols] per (block, seg); 4 regions
                    # per bank; accumulation groups are sequential per region.
                    nbank = _ceil(nb * 2, 4)
                    ps = [
                        pool.tile(
                            [128, 512], fp32, tag=f"edge{layer}_{i}",
                            name=f"ps{layer}_{i}",
                        )
                        for i in range(nbank)
                    ]

                    def psum_slice(b, seg):
                        j = (b - b0) * 2 + seg
                        return ps[j // 4][:, (j % 4) * 128 : (j % 4) * 128 + 128]

                    started = set()
                    left = {}
                    for b in range(b0, b0 + nb):
                        left[(b, 0)] = int(pl[g, b])
                        left[(b, 1)] = int(ph[g, b])

                    for (c0, ncols) in ((lo0, lon), (hi0, hin)):
                        seg = 0 if c0 == lo0 else 1
                        if layer == 1:
                            src_ap = (
                                xbf_in[g, :, :] if seg == 0 else xbf_in[g, SPLIT:, :]
                            )
                        else:
                            src_ap = (
                                hfull[g][:, :] if seg == 0 else hfull[g][SPLIT:, :]
                            )
                        for w0 in range(c0, c0 + ncols, WIN):
                            wn = min(WIN, c0 + ncols - w0)
                            gt = gpool.tile([128, WIN, 128], bf16, tag="gt")
                            nc.gpsimd.dma_gather(
                                gt[:, :wn, :],
                                src_ap,
                                idx_t[g][:, w0 * 8 : (w0 + wn) * 8],
                                wn * 128,
                                wn * 128,
                                128,
                                queue_num=qrr[0] % NQUEUE,
                            )
                            qrr[0] += 1
                            oh = opool.tile([128, WIN, 128], bf16, tag="oh")
                            nc.vector.tensor_tensor(
                                out=oh[:, :wn, :],
                                in0=dst_t[g][:, w0 : w0 + wn]
                                .rearrange("p (w o) -> p w o", o=1)
                                .to_broadcast([128, wn, 128]),
                                in1=iota_t[:]
                                .rearrange("p (o d) -> p o d", o=1)
                                .to_broadcast([128, wn, 128]),
                                op=mybir.AluOpType.is_equal,
                            )
                            for j in range(wn):
                                b, sseg = stream[g][w0 + j]
                                first = (b, sseg) not in started
                                if first:
                                    started.add((b, sseg))
                                left[(b, sseg)] -= 1
                                nc.tensor.matmul(
                                    out=psum_slice(b, sseg),
                                    lhsT=gt[:, j, :],
                                    rhs=oh[:, j, :],
                                    start=first,
                                    stop=left[(b, sseg)] == 0,
                                    skip_group_check=True,
                                )
                    yield b0, nb, psum_slice

            # ================= layer 1 (+AllGather) per graph =================
            for g in range(META):
                for b0, nb, psl in edge_phase(g, 1):
                    nw = nb * 128
                    # per-group streamed xts/invb slices
                    xtg = dpool.tile([128, GRP * 128], bf16, tag="xtg")
                    nc.sync.dma_start(
                        out=xtg[:, :nw], in_=xts_in[g, :, b0 * 128 : b0 * 128 + nw]
                    )
                    ibg = dpool.tile([128, GRP * 128], bf16, tag="ibg")
                    nc.sync.dma_start(
                        out=ibg[:, :nw], in_=invb_in[g, :, b0 * 128 : b0 * 128 + nw]
                    )
                    # meanT per block -> m1grp bf16 [F, nb*128]
                    m1g = dpool.tile([F, GRP, 128], bf16, tag="m1g")
                    for bi in range(nb):
                        b = b0 + bi
                        mc = dpool.tile([F, 128], fp32, tag="mc")
                        nc.vector.tensor_copy(out=mc[:], in_=psl(b, 0))
                        s0 = dpool.tile([F, 128], bf16, tag="s0")
                        nc.vector.tensor_tensor(
                            out=s0[:], in0=psl(b, 1), in1=mc[:],
                            op=mybir.AluOpType.add,
                        )
                        nc.vector.tensor_tensor(
                            out=m1g[:, bi, :],
                            in0=s0[:],
                            in1=ibg[:, bi * 128 : bi * 128 + 128],
                            op=mybir.AluOpType.mult,
                        )
                    # dense: o1 = W1l^T @ mean1T + W1r^T @ xT  [H, nb*128]
                    o1 = psO.tile([H, 512], fp32, tag="o1", name="o1")
                    nc.tensor.matmul(
                        out=o1[:, :nw],
                        lhsT=w1l_t[g][:],
                        rhs=m1g[:, :nb, :],
                        start=True,
                        stop=False,
                    )
                    nc.tensor.matmul(
                        out=o1[:, :nw],
                        lhsT=w1r_t[g][:],
                        rhs=xtg[:, :nw],
                        start=False,
                        stop=True,
                    )
                    # ELU: h = max(z,0) + exp(min(z,0)) - 1, z = o1 + b1
                    tm = dpool.tile([H, 512], fp32, tag="tm")
                    nc.vector.tensor_scalar(
                        out=tm[:, :nw],
                        in0=o1[:, :nw],
                        scalar1=b1_t[g][:, :1],
                        scalar2=0.0,
                        op0=mybir.AluOpType.add,
                        op1=mybir.AluOpType.min,
                    )
                    tp = dpool.tile([H, 512], fp32, tag="tp")
                    nc.vector.tensor_scalar(
                        out=tp[:, :nw],
                        in0=o1[:, :nw],
                        scalar1=b1_t[g][:, :1],
                        scalar2=0.0,
                        op0=mybir.AluOpType.add,
                        op1=mybir.AluOpType.max,
                    )
                    te = dpool.tile([H, 512], fp32, tag="te")
                    nc.scalar.activation(
                        out=te[:, :nw], in_=tm[:, :nw],
                        func=mybir.ActivationFunctionType.Exp,
                    )
                    ts_ = dpool.tile([H, 512], fp32, tag="ts")
                    nc.vector.tensor_tensor(
                        out=ts_[:, :nw], in0=te[:, :nw], in1=tp[:, :nw],
                        op=mybir.AluOpType.add,
                    )
                    hs = hT[g][:, b0 * 128 : b0 * 128 + nw]
                    nc.vector.tensor_scalar(
                        out=hs,
                        in0=ts_[:, :nw],
                        scalar1=-1.0,
                        scalar2=None,
                        op0=mybir.AluOpType.add,
                    )
                    # h rows (bf16) via PE transpose per block; one DMA per grp
                    hr = qpool.tile([128, GRP, 128], bf16, tag="hr")
                    for bi in range(nb):
                        b = b0 + bi
                        htr = psX.tile([128, 128], bf16, tag="work", name="htr")
                        nc.tensor.transpose(
                            out=htr[:],
                            in_=hT[g][:, b * 128 : b * 128 + 128],
                            identity=ident_t[:],
                        )
                        nc.vector.tensor_copy(out=hr[:, bi, :], in_=htr[:])
                    nc.sync.dma_start(
                        out=hshard[g][b0 * 128 : b0 * 128 + nw, :].rearrange(
                            "(b p) d -> p b d", p=128
                        ),
                        in_=hr[:, :nb, :],
                    )

                # ---- allgather h rows ----
                nc.gpsimd.collective_compute(
                    "AllGather",
                    mybir.AluOpType.bypass,
                    ins=[hshard[g][:]],
                    outs=[hfull[g][:]],
                    replica_groups=[core_ids],
                )

            # ================= layer 2, all graphs =================
            for g in range(META):
                for b0, nb, psl in edge_phase(g, 2):
                    nw = nb * 128
                    ibg = dpool.tile([128, GRP * 128], bf16, tag="ibg2")
                    nc.sync.dma_start(
                        out=ibg[:, :nw], in_=invb_in[g, :, b0 * 128 : b0 * 128 + nw]
                    )
                    og = qpool.tile([128, GRP, D], fp32, tag="og")
                    for bi in range(nb):
                        b = b0 + bi
                        mc = dpool.tile([H, 128], fp32, tag="mc2")
                        nc.vector.tensor_copy(out=mc[:], in_=psl(b, 0))
                        s2 = dpool.tile([H, 128], bf16, tag="s2")
                        nc.vector.tensor_tensor(
                            out=s2[:], in0=psl(b, 1), in1=mc[:],
                            op=mybir.AluOpType.add,
                        )
                        m2 = dpool.tile([H, 128], bf16, tag="m2")
                        nc.vector.tensor_tensor(
                            out=m2[:],
                            in0=s2[:],
                            in1=ibg[:, bi * 128 : bi * 128 + 128],
                            op=mybir.AluOpType.mult,
                        )
                        # o2 = mean2 @ W2l + h @ W2r  [128dst, D]
                        o2 = psX.tile([128, D], fp32, tag="work", name="o2")
                        nc.tensor.matmul(
                            out=o2[:], lhsT=m2[:], rhs=w2l_t[g][:],
                            start=True, stop=False,
                        )
                        nc.tensor.matmul(
                            out=o2[:],
                            lhsT=hT[g][:, b * 128 : b * 128 + 128],
                            rhs=w2r_t[g][:],
                            start=False,
                            stop=True,
                        )
                        nc.vector.tensor_tensor(
                            out=og[:, bi, :], in0=o2[:], in1=b2_t[g][:],
                            op=mybir.AluOpType.add,
                        )
                    # batched log_softmax along D for the whole group
                    rmax = dpool.tile([128, GRP, 1], fp32, tag="rmax")
                    nc.vector.reduce_max(
                        out=rmax[:, :nb, :], in_=og[:, :nb, :],
                        axis=mybir.AxisListType.X,
                    )
                    x1 = qpool.tile([128, GRP, D], fp32, tag="x1")
                    nc.vector.tensor_tensor(
                        out=x1[:, :nb, :],
                        in0=og[:, :nb, :],
                        in1=rmax[:, :nb, :].to_broadcast([128, nb, D]),
                        op=mybir.AluOpType.subtract,
                    )
                    ex = qpool.tile([128, GRP, D], fp32, tag="ex")
                    nc.scalar.activation(
                        out=ex[:, :nb, :], in_=x1[:, :nb, :],
                        func=mybir.ActivationFunctionType.Exp,
                    )
                    sm = dpool.tile([128, GRP, 1], fp32, tag="sm")
                    nc.vector.reduce_sum(
                        out=sm[:, :nb, :], in_=ex[:, :nb, :],
                        axis=mybir.AxisListType.X,
                    )
                    ls = dpool.tile([128, GRP, 1], fp32, tag="ls")
                    nc.scalar.activation(
                        out=ls[:, :nb, :], in_=sm[:, :nb, :],
                        func=mybir.ActivationFunctionType.Ln,
                    )
                    ob = qpool.tile([128, GRP, D], fp32, tag="ob")
                    nc.vector.tensor_tensor(
                        out=ob[:, :nb, :],
                        in0=x1[:, :nb, :],
                        in1=ls[:, :nb, :].to_broadcast([128, nb, D]),
                        op=mybir.AluOpType.subtract,
                    )
                    nc.sync.dma_start(
                        out=out_ext[g, b0 * 128 : b0 * 128 + nw, :].rearrange(
                            "(b p) d -> p b d", p=128
                        ),
                        in_=ob[:, :nb, :],
                    )

    nc.finalize()
    return nc


def kernel(**inputs):
    out, _ = run_kernel(inputs)
    return out


def run_kernel(inputs, trace=False):
    from concourse.bass_utils import run_bass_kernel_spmd

    meta_x = np.asarray(inputs["meta_x"], dtype=np.float32)
    layout, per_core = _prep_host(meta_x, inputs["meta_edge_index"])
    nc = _build_program(layout)

    xbf = meta_x.astype(BF16)
    iota = np.tile(np.arange(128, dtype=np.float32), (128, 1)).astype(BF16)
    ident = np.eye(128, dtype=np.float32).astype(BF16)
    w1l = np.asarray(inputs["W1l"], dtype=np.float32).astype(BF16)
    w1r = np.asarray(inputs["W1r"], dtype=np.float32).astype(BF16)
    w2l = np.asarray(inputs["W2l"], dtype=np.float32).astype(BF16)
    w2r = np.asarray(inputs["W2r"], dtype=np.float32).astype(BF16)
    b1 = np.asarray(inputs["b1"], dtype=np.float32)
    b2 = np.asarray(inputs["b2"], dtype=np.float32)
    b1c = b1[:, :, None].copy()
    b2b = np.broadcast_to(b2[:, None, :], (META, 128, D)).copy()

    in_maps = []
    for c in range(NCORES):
        pc = per_core[c]
        in_maps.append(
            {
                "xbf": xbf,
                "idx": pc["idxw"],
                "dstw": pc["dstw"],
                "xts": pc["xts"],
                "invb": pc["invb"],
                "w1l": w1l,
                "w1r": w1r,
                "b1c": b1c,
                "w2l": w2l,
                "w2r": w2r,
                "b2b": b2b,
                "iota": iota,
                "ident": ident,
            }
        )

    res = run_bass_kernel_spmd(nc, in_maps, list(range(NCORES)), trace=trace)
    out = np.zeros((META, N, D), dtype=np.float32)
    for c in range(NCORES):
        lo = c * NSH
        n = min(NSH, N - lo)
        out[:, lo : lo + n, :] = res.results[c]["out"][:, :n, :]
    return out, res


# revision 16
# speedup vs baseline: 1.0213x; 1.0213x over previous
"""MetaGraphSAGE Trainium2 kernel (8 NeuronCores, Bass/Tile), bf16 edge path.

Per metagraph (3 independent graphs):
    h  = ELU(mean_agg(x) @ W1l + x @ W1r + b1)
    o  = mean_agg(h) @ W2l + h @ W2r + b2
    out = log_softmax(o, axis=1)

Sharding: nodes padded to 8*6272=50176; core c owns dst nodes
[c*6272,(c+1)*6272). Edges are partitioned by dst owner on the host and
sorted by 128-wide dst block. On device, messages are gathered with
dma_gather (256B bf16 rows) and segment-summed via bf16 one-hot matmuls
accumulating in fp32 PSUM as aggT [128feat x 128dst] for BOTH layers.
Layer 2 gathers bf16 h rows from an in-kernel AllGather output (h rows
are produced per block by a PE transpose of the ELU output). int16
gather indices force a lo/hi split of each block-group's edge list at
src=32768. Dense matmuls and ELU batch over GRP=4 blocks; log_softmax
batches over each group to limit activation-table thrash.
"""

import sys

sys.path.insert(0, "/opt/trn_rl_repo")

import ml_dtypes
import numpy as np

META, N, E, F, H, D = 3, 50000, 640000, 128, 128, 64
NCORES = 8
NSH = 6272            # padded nodes per core (6272*8 = 50176 >= N)
NPAD = NSH * NCORES
NBLK = NSH // 128     # 49 blocks of 128 dst nodes per core
GRP = 4               # dst blocks per psum bank-pair / dense batch
WIN = 5               # max 128-edge chunks per dma_gather call (HW cap ~1024 idx)
SPLIT = 32768         # int16 gather index limit
DMA_SCRATCH = 16384   # per-partition SWDGE descriptor carveout bytes
NQUEUE = 4            # SWDGE queues for gather round-robin (ucode max)

BF16 = ml_dtypes.bfloat16


def _ceil(a, b):
    return (a + b - 1) // b


def _ceil_arr(a, b):
    return (a + b - 1) // b


def _prep_host(meta_x, meta_edge_index):
    """Partition+sort edges per (core, graph); build all per-core arrays."""
    meta_x = np.asarray(meta_x, dtype=np.float32)
    ei = np.asarray(meta_edge_index, dtype=np.int64)

    counts_lo = np.zeros((NCORES, META, NBLK), dtype=np.int64)
    counts_hi = np.zeros((NCORES, META, NBLK), dtype=np.int64)
    edata = {}  # (c,g,blk,seg) -> (src_idx_array, d128_array)
    inv_all = np.zeros((NCORES, META, NSH), dtype=np.float32)
    for g in range(META):
        src = ei[g, 0]
        dst = ei[g, 1]
        core = dst // NSH
        for c in range(NCORES):
            sel = core == c
            s = src[sel]
            dl = dst[sel] - c * NSH
            cnt = np.bincount(dl, minlength=NSH).astype(np.float32)
            inv_all[c, g] = 1.0 / np.maximum(cnt, 1.0)
            blk = dl // 128
            d128 = dl % 128
            hi = (s >= SPLIT).astype(np.int64)
            order = np.lexsort((hi, blk))
            s, d128, blk, hi = s[order], d128[order], blk[order], hi[order]
            for b in range(NBLK):
                mb = blk == b
                sb, db, hb = s[mb], d128[mb], hi[mb]
                lo_n = int((hb == 0).sum())
                counts_lo[c, g, b] = lo_n
                counts_hi[c, g, b] = len(sb) - lo_n
                edata[(c, g, b, 0)] = (sb[:lo_n], db[:lo_n])
                edata[(c, g, b, 1)] = (sb[lo_n:] - SPLIT, db[lo_n:])

    # padded chunk counts per (g, blk, seg): max over cores, in 128-chunks
    pl = np.maximum(_ceil_arr(counts_lo.max(axis=0), 128), 1)  # [META,NBLK]
    ph = np.maximum(_ceil_arr(counts_hi.max(axis=0), 128), 1)

    # stream layout per graph: groups of GRP blocks, [lo segs | hi segs]
    stream = [[] for _ in range(META)]   # per g: list of (blk, seg)
    groups = []  # (g, blk0, nblk, lo_chunk0, lo_nchunk, hi_chunk0, hi_nchunk)
    for g in range(META):
        for b0 in range(0, NBLK, GRP):
            nb = min(GRP, NBLK - b0)
            lo0 = len(stream[g])
            for b in range(b0, b0 + nb):
                stream[g] += [(b, 0)] * int(pl[g, b])
            hi0 = len(stream[g])
            for b in range(b0, b0 + nb):
                stream[g] += [(b, 1)] * int(ph[g, b])
            groups.append((g, b0, nb, lo0, hi0 - lo0, hi0, len(stream[g]) - hi0))
    totc = [len(stream[g]) for g in range(META)]  # chunks per graph

    # per-core flat idx/dst arrays in stream order (shared by both layers)
    per_core = []
    for c in range(NCORES):
        idx16 = [np.zeros((t * 128,), dtype=np.int16) for t in totc]
        dstf = [np.full((t * 128,), -1.0, dtype=np.float32) for t in totc]
        for g in range(META):
            pos = 0
            cur = None
            for (b, seg) in stream[g]:
                if cur != (b, seg):
                    cur = (b, seg)
                    sarr, darr = edata[(c, g, b, seg)]
                    off = 0
                n = min(128, len(sarr) - off) if len(sarr) > off else 0
                if n > 0:
                    idx16[g][pos : pos + n] = sarr[off : off + n].astype(np.int16)
                    dstf[g][pos : pos + n] = darr[off : off + n].astype(np.float32)
                off += 128
                pos += 128
        idxw = np.concatenate(
            [np.tile(a.reshape(-1, 16).T, (8, 1)) for a in idx16], axis=1
        )  # [128, sum(totc)*8]
        dstw = np.concatenate(
            [a.reshape(-1, 128).T for a in dstf], axis=1
        ).astype(BF16)  # [128, sum(totc)]
        xts = np.zeros((META, 128, NSH), dtype=BF16)
        lo = c * NSH
        hi_n = min(NSH, N - lo)
        for g in range(META):
            if hi_n > 0:
                xts[g, :, :hi_n] = meta_x[g, lo : lo + hi_n].T.astype(BF16)
        invb = np.broadcast_to(
            inv_all[c][:, None, :], (META, 128, NSH)
        ).astype(BF16)  # [META,128,NSH]
        per_core.append(dict(idxw=idxw, dstw=dstw, xts=xts, invb=invb))

    layout = dict(stream=stream, groups=groups, totc=totc, pl=pl, ph=ph)
    return layout, per_core


def _build_program(layout):
    import concourse.bass as bass  # noqa: F401
    import concourse.mybir as mybir
    import concourse.tile as tile
    from concourse import bacc

    fp32 = mybir.dt.float32
    bf16 = mybir.dt.bfloat16
    i16 = mybir.dt.int16
    nc = bacc.Bacc(None, dynamic_dma_scratch_size=DMA_SCRATCH, num_swdge_queues=NQUEUE)
    core_ids = list(range(NCORES))

    totc = layout["totc"]
    totc_all = sum(totc)

    # ---- dram I/O ----
    xbf_in = nc.declare_dram_parameter("xbf", [META, N, F], bf16, isOutput=False)
    idx_in = nc.declare_dram_parameter("idx", [128, totc_all * 8], i16, isOutput=False)
    dst_in = nc.declare_dram_parameter("dstw", [128, totc_all], bf16, isOutput=False)
    xts_in = nc.declare_dram_parameter("xts", [META, 128, NSH], bf16, isOutput=False)
    invb_in = nc.declare_dram_parameter("invb", [META, 128, NSH], bf16, isOutput=False)
    w1l_in = nc.declare_dram_parameter("w1l", [META, F, H], bf16, isOutput=False)
    w1r_in = nc.declare_dram_parameter("w1r", [META, F, H], bf16, isOutput=False)
    b1_in = nc.declare_dram_parameter("b1c", [META, H, 1], fp32, isOutput=False)
    b1n_in = nc.declare_dram_parameter("b1n", [META, H, 1], fp32, isOutput=False)
    w2l_in = nc.declare_dram_parameter("w2l", [META, H, D], bf16, isOutput=False)
    w2r_in = nc.declare_dram_parameter("w2r", [META, H, D], bf16, isOutput=False)
    b2_in = nc.declare_dram_parameter("b2b", [META, 128, D], fp32, isOutput=False)
    iota_in = nc.declare_dram_parameter("iota", [128, 128], bf16, isOutput=False)
    ident_in = nc.declare_dram_parameter("ident", [128, 128], bf16, isOutput=False)
    out_ext = nc.declare_dram_parameter("out", [META, NSH, D], fp32, isOutput=True)

    hshard = [nc.dram_tensor(f"hshard{g}", [NSH, H], bf16) for g in range(META)]
    hfull = [
        nc.dram_tensor(f"hfull{g}", [NPAD, H], bf16, addr_space="Shared")
        for g in range(META)
    ]

    stream = layout["stream"]
    groups = layout["groups"]
    pl, ph = layout["pl"], layout["ph"]
    goff = [sum(totc[:g]) for g in range(META)]

    with tile.TileContext(nc) as tc:
        with (
            tc.tile_pool(name="const", bufs=1) as cpool,
            tc.tile_pool(name="weights", bufs=1) as wpool,
            tc.tile_pool(name="hT", bufs=1) as hpool,
            tc.tile_pool(name="meta", bufs=1) as mpool,
            tc.tile_pool(name="gath", bufs=6) as gpool,
            tc.tile_pool(name="oneh", bufs=6) as opool,
            tc.tile_pool(name="grp", bufs=2) as qpool,
            tc.tile_pool(name="dense", bufs=3) as dpool,
            tc.tile_pool(name="psA", bufs=1, space="PSUM") as psA,
            tc.tile_pool(name="psD", bufs=1, space="PSUM") as psD,
            tc.tile_pool(name="psO", bufs=2, space="PSUM") as psO,
            tc.tile_pool(name="psX", bufs=2, space="PSUM") as psX,
        ):
            iota_t = cpool.tile([128, 128], bf16, tag="iota")
            nc.sync.dma_start(out=iota_t[:], in_=iota_in[:])
            ident_t = cpool.tile([128, 128], bf16, tag="ident")
            nc.sync.dma_start(out=ident_t[:], in_=ident_in[:])

            # resident per-graph constants + metadata
            w1l_t, w1r_t, w2l_t, w2r_t, b1_t, b1n_t, b2_t = [], [], [], [], [], [], []
            idx_t, dst_t = [], []
            for g in range(META):
                t = wpool.tile([F, H], bf16, tag=f"w1l{g}", name=f"w1l{g}")
                nc.sync.dma_start(out=t[:], in_=w1l_in[g])
                w1l_t.append(t)
                t = wpool.tile([F, H], bf16, tag=f"w1r{g}", name=f"w1r{g}")
                nc.sync.dma_start(out=t[:], in_=w1r_in[g])
                w1r_t.append(t)
                t = wpool.tile([H, D], bf16, tag=f"w2l{g}", name=f"w2l{g}")
                nc.sync.dma_start(out=t[:], in_=w2l_in[g])
                w2l_t.append(t)
                t = wpool.tile([H, D], bf16, tag=f"w2r{g}", name=f"w2r{g}")
                nc.sync.dma_start(out=t[:], in_=w2r_in[g])
                w2r_t.append(t)
                t = wpool.tile([H, 1], fp32, tag=f"b1{g}", name=f"b1{g}")
                nc.sync.dma_start(out=t[:], in_=b1_in[g])
                b1_t.append(t)
                t = wpool.tile([H, 1], fp32, tag=f"b1n{g}", name=f"b1n{g}")
                nc.sync.dma_start(out=t[:], in_=b1n_in[g])
                b1n_t.append(t)
                t = wpool.tile([128, D], fp32, tag=f"b2{g}", name=f"b2{g}")
                nc.sync.dma_start(out=t[:], in_=b2_in[g])
                b2_t.append(t)
                t = mpool.tile([128, totc[g] * 8], i16, tag=f"idx{g}", name=f"idx{g}")
                nc.sync.dma_start(
                    out=t[:], in_=idx_in[:, goff[g] * 8 : (goff[g] + totc[g]) * 8]
                )
                idx_t.append(t)
                t = mpool.tile([128, totc[g]], bf16, tag=f"dst{g}", name=f"dst{g}")
                nc.sync.dma_start(
                    out=t[:], in_=dst_in[:, goff[g] : goff[g] + totc[g]]
                )
                dst_t.append(t)

            hT = [
                hpool.tile([H, NSH], bf16, tag=f"hTg{g}", name=f"hTg{g}")
                for g in range(META)
            ]

            qrr = [0]

            def edge_phase(g, layer):
                """Gather + one-hot matmul accumulate for all groups of graph g.

                Both layers produce psum aggT [128feat x 128dst] per (block,
                seg) via lhsT=gathered bf16 rows, rhs=bf16 one-hot. Yields
                (b0, nb, psum_slice) per group once its psums are complete.
                """
                pool = psA if layer == 1 else psD
                for (gg, b0, nb, lo0, lon, hi0, hin) in groups:
                    if gg != g:
                        continue
                    # one psum region [128 cols] per (block, seg); 4 regions
                    # per bank; accumulation groups are sequential per region.
                    nbank = _ceil(nb * 2, 4)
                    ps = [
                        pool.tile(
                            [128, 512], fp32, tag=f"edge{layer}_{i}",
                            name=f"ps{layer}_{i}",
                        )
                        for i in range(nbank)
                    ]

                    def psum_slice(b, seg):
                        j = (b - b0) * 2 + seg
                        return ps[j // 4][:, (j % 4) * 128 : (j % 4) * 128 + 128]

                    started = set()
                    left = {}
                    for b in range(b0, b0 + nb):
                        left[(b, 0)] = int(pl[g, b])
                        left[(b, 1)] = int(ph[g, b])

                    for (c0, ncols) in ((lo0, lon), (hi0, hin)):
                        seg = 0 if c0 == lo0 else 1
                        if layer == 1:
                            src_ap = (
                                xbf_in[g, :, :] if seg == 0 else xbf_in[g, SPLIT:, :]
                            )
                        else:
                            src_ap = (
                                hfull[g][:, :] if seg == 0 else hfull[g][SPLIT:, :]
                            )
                        for w0 in range(c0, c0 + ncols, WIN):
                            wn = min(WIN, c0 + ncols - w0)
                            gt = gpool.tile([128, WIN, 128], bf16, tag="gt")
                            nc.gpsimd.dma_gather(
                                gt[:, :wn, :],
                                src_ap,
                                idx_t[g][:, w0 * 8 : (w0 + wn) * 8],
                                wn * 128,
                                wn * 128,
                                128,
                                queue_num=qrr[0] % NQUEUE,
                            )
                            qrr[0] += 1
                            oh = opool.tile([128, WIN, 128], bf16, tag="oh")
                            nc.vector.tensor_tensor(
                                out=oh[:, :wn, :],
                                in0=dst_t[g][:, w0 : w0 + wn]
                                .rearrange("p (w o) -> p w o", o=1)
                                .to_broadcast([128, wn, 128]),
                                in1=iota_t[:]
                                .rearrange("p (o d) -> p o d", o=1)
                                .to_broadcast([128, wn, 128]),
                                op=mybir.AluOpType.is_equal,
                            )
                            for j in range(wn):
                                b, sseg = stream[g][w0 + j]
                                first = (b, sseg) not in started
                                if first:
                                    started.add((b, sseg))
                                left[(b, sseg)] -= 1
                                nc.tensor.matmul(
                                    out=psum_slice(b, sseg),
                                    lhsT=gt[:, j, :],
                                    rhs=oh[:, j, :],
                                    start=first,
                                    stop=left[(b, sseg)] == 0,
                                    skip_group_check=True,
                                )
                    yield b0, nb, psum_slice

            # ================= layer 1 (+AllGather) per graph =================
            for g in range(META):
                for b0, nb, psl in edge_phase(g, 1):
                    nw = nb * 128
                    # per-group streamed xts/invb slices
                    xtg = dpool.tile([128, GRP * 128], bf16, tag="xtg")
                    nc.sync.dma_start(
                        out=xtg[:, :nw], in_=xts_in[g, :, b0 * 128 : b0 * 128 + nw]
                    )
                    ibg = dpool.tile([128, GRP * 128], bf16, tag="ibg")
                    nc.sync.dma_start(
                        out=ibg[:, :nw], in_=invb_in[g, :, b0 * 128 : b0 * 128 + nw]
                    )
                    # meanT per block -> m1grp bf16 [F, nb*128]
                    m1g = dpool.tile([F, GRP, 128], bf16, tag="m1g")
                    for bi in range(nb):
                        b = b0 + bi
                        mc = dpool.tile([F, 128], fp32, tag="mc")
                        nc.vector.tensor_copy(out=mc[:], in_=psl(b, 0))
                        s0 = dpool.tile([F, 128], bf16, tag="s0")
                        nc.vector.tensor_tensor(
                            out=s0[:], in0=psl(b, 1), in1=mc[:],
                            op=mybir.AluOpType.add,
                        )
                        nc.vector.tensor_tensor(
                            out=m1g[:, bi, :],
                            in0=s0[:],
                            in1=ibg[:, bi * 128 : bi * 128 + 128],
                            op=mybir.AluOpType.mult,
                        )
                    # dense: o1 = W1l^T @ mean1T + W1r^T @ xT  [H, nb*128]
                    o1 = psO.tile([H, 512], fp32, tag="o1", name="o1")
                    nc.tensor.matmul(
                        out=o1[:, :nw],
                        lhsT=w1l_t[g][:],
                        rhs=m1g[:, :nb, :],
                        start=True,
                        stop=False,
                    )
                    nc.tensor.matmul(
                        out=o1[:, :nw],
                        lhsT=w1r_t[g][:],
                        rhs=xtg[:, :nw],
                        start=False,
                        stop=True,
                    )
                    # ELU: h = relu(z) + exp(-relu(-z)) - 1, z = o1 + b1.
                    # relu/exp/identity(+bias) run on the scalar engine to
                    # keep DVE off the critical path.
                    tp = dpool.tile([H, 512], fp32, tag="tp")
                    nc.scalar.activation(
                        out=tp[:, :nw], in_=o1[:, :nw],
                        func=mybir.ActivationFunctionType.Relu,
                        bias=b1_t[g][:, :1],
                    )
                    tm = dpool.tile([H, 512], fp32, tag="tm")
                    nc.scalar.activation(
                        out=tm[:, :nw], in_=o1[:, :nw],
                        func=mybir.ActivationFunctionType.Relu,
                        bias=b1n_t[g][:, :1], scale=-1.0,
                    )
                    te = dpool.tile([H, 512], fp32, tag="te")
                    nc.scalar.activation(
                        out=te[:, :nw], in_=tm[:, :nw],
                        func=mybir.ActivationFunctionType.Exp, scale=-1.0,
                    )
                    ts_ = dpool.tile([H, 512], fp32, tag="ts")
                    nc.vector.tensor_tensor(
                        out=ts_[:, :nw], in0=te[:, :nw], in1=tp[:, :nw],
                        op=mybir.AluOpType.add,
                    )
                    hs = hT[g][:, b0 * 128 : b0 * 128 + nw]
                    nc.scalar.activation(
                        out=hs, in_=ts_[:, :nw],
                        func=mybir.ActivationFunctionType.Identity, bias=-1.0,
                    )
                    # h rows (bf16) via PE transpose per block; one DMA per grp
                    hr = qpool.tile([128, GRP, 128], bf16, tag="hr")
                    for bi in range(nb):
                        b = b0 + bi
                        htr = psX.tile([128, 128], bf16, tag="work", name="htr")
                        nc.tensor.transpose(
                            out=htr[:],
                            in_=hT[g][:, b * 128 : b * 128 + 128],
                            identity=ident_t[:],
                        )
                        nc.vector.tensor_copy(out=hr[:, bi, :], in_=htr[:])
                    nc.sync.dma_start(
                        out=hshard[g][b0 * 128 : b0 * 128 + nw, :].rearrange(
                            "(b p) d -> p b d", p=128
                        ),
                        in_=hr[:, :nb, :],
                    )

                # ---- allgather h rows ----
                nc.gpsimd.collective_compute(
                    "AllGather",
                    mybir.AluOpType.bypass,
                    ins=[hshard[g][:]],
                    outs=[hfull[g][:]],
                    replica_groups=[core_ids],
                )

            # ================= layer 2, all graphs =================
            for g in range(META):
                for b0, nb, psl in edge_phase(g, 2):
                    nw = nb * 128
                    ibg = dpool.tile([128, GRP * 128], bf16, tag="ibg2")
                    nc.sync.dma_start(
                        out=ibg[:, :nw], in_=invb_in[g, :, b0 * 128 : b0 * 128 + nw]
                    )
                    og = qpool.tile([128, GRP, D], fp32, tag="og")
                    for bi in range(nb):
                        b = b0 + bi
                        mc = dpool.tile([H, 128], fp32, tag="mc2")
                        nc.vector.tensor_copy(out=mc[:], in_=psl(b, 0))
                        s2 = dpool.tile([H, 128], bf16, tag="s2")
                        nc.vector.tensor_tensor(
                            out=s2[:], in0=psl(b, 1), in1=mc[:],
                            op=mybir.AluOpType.add,
                        )
                        m2 = dpool.tile([H, 128], bf16, tag="m2")
                        nc.vector.tensor_tensor(
                            out=m2[:],
                            in0=s2[:],
                            in1=ibg[:, bi * 128 : bi * 128 + 128],
                            op=mybir.AluOpType.mult,
                        )
                        # o2 = mean2 @ W2l + h @ W2r  [128dst, D]
                        o2 = psX.tile([128, D], fp32, tag="work", name="o2")
                        nc.tensor.matmul(
                            out=o2[:], lhsT=m2[:], rhs=w2l_t[g][:],
                            start=True, stop=False,
                        )
                        nc.tensor.matmul(
                            out=o2[:],
                            lhsT=hT[g][:, b * 128 : b * 128 + 128],
                            rhs=w2r_t[g][:],
                            start=False,
                            stop=True,
                        )
                        nc.vector.tensor_tensor(
                            out=og[:, bi, :], in0=o2[:], in1=b2_t[g][:],
                            op=mybir.AluOpType.add,
                        )
                    # batched log_softmax along D for the whole group
                    rmax = dpool.tile([128, GRP, 1], fp32, tag="rmax")
                    nc.vector.reduce_max(
                        out=rmax[:, :nb, :], in_=og[:, :nb, :],
                        axis=mybir.AxisListType.X,
                    )
                    x1 = qpool.tile([128, GRP, D], fp32, tag="x1")
                    nc.vector.tensor_tensor(
                        out=x1[:, :nb, :],
                        in0=og[:, :nb, :],
                        in1=rmax[:, :nb, :].to_broadcast([128, nb, D]),
                        op=mybir.AluOpType.subtract,
                    )
                    ex = qpool.tile([128, GRP, D], fp32, tag="ex")
                    nc.scalar.activation(
                        out=ex[:, :nb, :], in_=x1[:, :nb, :],
                        func=mybir.ActivationFunctionType.Exp,
                    )
                    sm = dpool.tile([128, GRP, 1], fp32, tag="sm")
                    nc.vector.reduce_sum(
                        out=sm[:, :nb, :], in_=ex[:, :nb, :],
                        axis=mybir.AxisListType.X,
                    )
                    ls = dpool.tile([128, GRP, 1], fp32, tag="ls")
                    nc.scalar.activation(
                        out=ls[:, :nb, :], in_=sm[:, :nb, :],
                        func=mybir.ActivationFunctionType.Ln,
                    )
                    ob = qpool.tile([128, GRP, D], fp32, tag="ob")
                    nc.vector.tensor_tensor(
                        out=ob[:, :nb, :],
                        in0=x1[:, :nb, :],
                        in1=ls[:, :nb, :].to_broadcast([128, nb, D]),
                        op=mybir.AluOpType.subtract,
                    )
                    nc.sync.dma_start(
                        out=out_ext[g, b0 * 128 : b0 * 128 + nw, :].rearrange(
                            "(b p) d -> p b d", p=128
                        ),
                        in_=ob[:, :nb, :],
                    )

    nc.finalize()
    return nc


def kernel(**inputs):
    out, _ = run_kernel(inputs)
    return out


def run_kernel(inputs, trace=False):
    from concourse.bass_utils import run_bass_kernel_spmd

    meta_x = np.asarray(inputs["meta_x"], dtype=np.float32)
    layout, per_core = _prep_host(meta_x, inputs["meta_edge_index"])
    nc = _build_program(layout)

    xbf = meta_x.astype(BF16)
    iota = np.tile(np.arange(128, dtype=np.float32), (128, 1)).astype(BF16)
    ident = np.eye(128, dtype=np.float32).astype(BF16)
    w1l = np.asarray(inputs["W1l"], dtype=np.float32).astype(BF16)
    w1r = np.asarray(inputs["W1r"], dtype=np.float32).astype(BF16)
    w2l = np.asarray(inputs["W2l"], dtype=np.float32).astype(BF16)
    w2r = np.asarray(inputs["W2r"], dtype=np.float32).astype(BF16)
    b1 = np.asarray(inputs["b1"], dtype=np.float32)
    b2 = np.asarray(inputs["b2"], dtype=np.float32)
    b1c = b1[:, :, None].copy()
    b1nc = -b1c
    b2b = np.broadcast_to(b2[:, None, :], (META, 128, D)).copy()

    in_maps = []
    for c in range(NCORES):
        pc = per_core[c]
        in_maps.append(
            {
                "xbf": xbf,
                "idx": pc["idxw"],
                "dstw": pc["dstw"],
                "xts": pc["xts"],
                "invb": pc["invb"],
                "w1l": w1l,
                "w1r": w1r,
                "b1c": b1c,
                "b1n": b1nc,
                "w2l": w2l,
                "w2r": w2r,
                "b2b": b2b,
                "iota": iota,
                "ident": ident,
            }
        )

    res = run_bass_kernel_spmd(nc, in_maps, list(range(NCORES)), trace=trace)
    out = np.zeros((META, N, D), dtype=np.float32)
    for c in range(NCORES):
        lo = c * NSH
        n = min(NSH, N - lo)
        out[:, lo : lo + n, :] = res.results[c]["out"][:, :n, :]
    return out, res


# revision 18
# speedup vs baseline: 1.1208x; 1.0974x over previous
"""MetaGraphSAGE Trainium2 kernel (8 NeuronCores, Bass/Tile), bf16 edge path.

Per metagraph (3 independent graphs):
    h  = ELU(mean_agg(x) @ W1l + x @ W1r + b1)
    o  = mean_agg(h) @ W2l + h @ W2r + b2
    out = log_softmax(o, axis=1)

Sharding: nodes padded to 8*6272=50176; core c owns dst nodes
[c*6272,(c+1)*6272). Edges are partitioned by dst owner on the host and
sorted by 128-wide dst block. On device, messages are gathered with
dma_gather (256B bf16 rows) and segment-summed via bf16 one-hot matmuls
accumulating in fp32 PSUM as aggT [128feat x 128dst] for BOTH layers.
Layer 2 gathers bf16 h rows from an in-kernel AllGather output (h rows
are produced per block by a PE transpose of the ELU output). int16
gather indices force a lo/hi split of each block-group's edge list at
src=32768. Dense matmuls and ELU batch over GRP=4 blocks; log_softmax
batches over each group to limit activation-table thrash.
"""

import sys

sys.path.insert(0, "/opt/trn_rl_repo")

import ml_dtypes
import numpy as np

META, N, E, F, H, D = 3, 50000, 640000, 128, 128, 64
NCORES = 8
NSH = 6272            # padded nodes per core (6272*8 = 50176 >= N)
NPAD = NSH * NCORES
NBLK = NSH // 128     # 49 blocks of 128 dst nodes per core
GRP = 4               # dst blocks per psum bank-pair / dense batch
WIN = 5               # max 128-edge chunks per dma_gather call (HW cap ~1024 idx)
SPLIT = 32768         # int16 gather index limit
DMA_SCRATCH = 16384   # per-partition SWDGE descriptor carveout bytes
NQUEUE = 4            # SWDGE queues for gather round-robin (ucode max)

BF16 = ml_dtypes.bfloat16


def _ceil(a, b):
    return (a + b - 1) // b


def _ceil_arr(a, b):
    return (a + b - 1) // b


def _prep_host(meta_x, meta_edge_index):
    """Partition+sort edges per (core, graph); build all per-core arrays."""
    meta_x = np.asarray(meta_x, dtype=np.float32)
    ei = np.asarray(meta_edge_index, dtype=np.int64)

    counts_lo = np.zeros((NCORES, META, NBLK), dtype=np.int64)
    counts_hi = np.zeros((NCORES, META, NBLK), dtype=np.int64)
    edata = {}  # (c,g,blk,seg) -> (src_idx_array, d128_array)
    inv_all = np.zeros((NCORES, META, NSH), dtype=np.float32)
    for g in range(META):
        src = ei[g, 0]
        dst = ei[g, 1]
        core = dst // NSH
        for c in range(NCORES):
            sel = core == c
            s = src[sel]
            dl = dst[sel] - c * NSH
            cnt = np.bincount(dl, minlength=NSH).astype(np.float32)
            inv_all[c, g] = 1.0 / np.maximum(cnt, 1.0)
            blk = dl // 128
            d128 = dl % 128
            hi = (s >= SPLIT).astype(np.int64)
            order = np.lexsort((hi, blk))
            s, d128, blk, hi = s[order], d128[order], blk[order], hi[order]
            for b in range(NBLK):
                mb = blk == b
                sb, db, hb = s[mb], d128[mb], hi[mb]
                lo_n = int((hb == 0).sum())
                counts_lo[c, g, b] = lo_n
                counts_hi[c, g, b] = len(sb) - lo_n
                edata[(c, g, b, 0)] = (sb[:lo_n], db[:lo_n])
                edata[(c, g, b, 1)] = (sb[lo_n:] - SPLIT, db[lo_n:])

    # padded chunk counts per (g, blk, seg): max over cores, in 128-chunks
    pl = np.maximum(_ceil_arr(counts_lo.max(axis=0), 128), 1)  # [META,NBLK]
    ph = np.maximum(_ceil_arr(counts_hi.max(axis=0), 128), 1)

    # stream layout per graph: groups of GRP blocks, [lo segs | hi segs]
    stream = [[] for _ in range(META)]   # per g: list of (blk, seg)
    groups = []  # (g, blk0, nblk, lo_chunk0, lo_nchunk, hi_chunk0, hi_nchunk)
    for g in range(META):
        for b0 in range(0, NBLK, GRP):
            nb = min(GRP, NBLK - b0)
            lo0 = len(stream[g])
            for b in range(b0, b0 + nb):
                stream[g] += [(b, 0)] * int(pl[g, b])
            hi0 = len(stream[g])
            for b in range(b0, b0 + nb):
                stream[g] += [(b, 1)] * int(ph[g, b])
            groups.append((g, b0, nb, lo0, hi0 - lo0, hi0, len(stream[g]) - hi0))
    totc = [len(stream[g]) for g in range(META)]  # chunks per graph

    # per-core flat idx/dst arrays in stream order (shared by both layers)
    per_core = []
    for c in range(NCORES):
        idx16 = [np.zeros((t * 128,), dtype=np.int16) for t in totc]
        dstf = [np.full((t * 128,), -1.0, dtype=np.float32) for t in totc]
        for g in range(META):
            pos = 0
            cur = None
            for (b, seg) in stream[g]:
                if cur != (b, seg):
                    cur = (b, seg)
                    sarr, darr = edata[(c, g, b, seg)]
                    off = 0
                n = min(128, len(sarr) - off) if len(sarr) > off else 0
                if n > 0:
                    idx16[g][pos : pos + n] = sarr[off : off + n].astype(np.int16)
                    dstf[g][pos : pos + n] = darr[off : off + n].astype(np.float32)
                off += 128
                pos += 128
        idxw = np.concatenate(
            [np.tile(a.reshape(-1, 16).T, (8, 1)) for a in idx16], axis=1
        )  # [128, sum(totc)*8]
        dstw = np.concatenate(
            [a.reshape(-1, 128).T for a in dstf], axis=1
        ).astype(BF16)  # [128, sum(totc)]
        xts = np.zeros((META, 128, NSH), dtype=BF16)
        lo = c * NSH
        hi_n = min(NSH, N - lo)
        for g in range(META):
            if hi_n > 0:
                xts[g, :, :hi_n] = meta_x[g, lo : lo + hi_n].T.astype(BF16)
        invb = np.broadcast_to(
            inv_all[c][:, None, :], (META, 128, NSH)
        ).astype(BF16)  # [META,128,NSH]
        per_core.append(dict(idxw=idxw, dstw=dstw, xts=xts, invb=invb))

    layout = dict(stream=stream, groups=groups, totc=totc, pl=pl, ph=ph)
    return layout, per_core


def _build_program(layout):
    import concourse.bass as bass  # noqa: F401
    import concourse.mybir as mybir
    import concourse.tile as tile
    from concourse import bacc

    fp32 = mybir.dt.float32
    bf16 = mybir.dt.bfloat16
    i16 = mybir.dt.int16
    nc = bacc.Bacc(None, dynamic_dma_scratch_size=DMA_SCRATCH, num_swdge_queues=NQUEUE)
    core_ids = list(range(NCORES))

    # scalar.activation float bias/scale lowers via registered const APs;
    # -1.0 is not in the default set.
    _cm1 = nc.alloc_sbuf_tensor("const-float32--1.0", [128, 1], fp32)
    nc.gpsimd.memset(_cm1.ap(), -1.0)
    nc.const_aps.aps[(fp32, -1.0)] = _cm1.ap()
    nc.all_engine_barrier()

    totc = layout["totc"]
    totc_all = sum(totc)

    # ---- dram I/O ----
    xbf_in = nc.declare_dram_parameter("xbf", [META, N, F], bf16, isOutput=False)
    idx_in = nc.declare_dram_parameter("idx", [128, totc_all * 8], i16, isOutput=False)
    dst_in = nc.declare_dram_parameter("dstw", [128, totc_all], bf16, isOutput=False)
    xts_in = nc.declare_dram_parameter("xts", [META, 128, NSH], bf16, isOutput=False)
    invb_in = nc.declare_dram_parameter("invb", [META, 128, NSH], bf16, isOutput=False)
    w1l_in = nc.declare_dram_parameter("w1l", [META, F, H], bf16, isOutput=False)
    w1r_in = nc.declare_dram_parameter("w1r", [META, F, H], bf16, isOutput=False)
    b1_in = nc.declare_dram_parameter("b1c", [META, H, 1], fp32, isOutput=False)
    b1n_in = nc.declare_dram_parameter("b1n", [META, H, 1], fp32, isOutput=False)
    w2l_in = nc.declare_dram_parameter("w2l", [META, H, D], bf16, isOutput=False)
    w2r_in = nc.declare_dram_parameter("w2r", [META, H, D], bf16, isOutput=False)
    b2_in = nc.declare_dram_parameter("b2b", [META, 128, D], fp32, isOutput=False)
    iota_in = nc.declare_dram_parameter("iota", [128, 128], bf16, isOutput=False)
    ident_in = nc.declare_dram_parameter("ident", [128, 128], bf16, isOutput=False)
    out_ext = nc.declare_dram_parameter("out", [META, NSH, D], fp32, isOutput=True)

    hshard = [nc.dram_tensor(f"hshard{g}", [NSH, H], bf16) for g in range(META)]
    hfull = [
        nc.dram_tensor(f"hfull{g}", [NPAD, H], bf16, addr_space="Shared")
        for g in range(META)
    ]

    stream = layout["stream"]
    groups = layout["groups"]
    pl, ph = layout["pl"], layout["ph"]
    goff = [sum(totc[:g]) for g in range(META)]

    with tile.TileContext(nc) as tc:
        with (
            tc.tile_pool(name="const", bufs=1) as cpool,
            tc.tile_pool(name="weights", bufs=1) as wpool,
            tc.tile_pool(name="hT", bufs=1) as hpool,
            tc.tile_pool(name="meta", bufs=1) as mpool,
            tc.tile_pool(name="gath", bufs=8) as gpool,
            tc.tile_pool(name="oneh", bufs=8) as opool,
            tc.tile_pool(name="grp", bufs=2) as qpool,
            tc.tile_pool(name="dense", bufs=3) as dpool,
            tc.tile_pool(name="psA", bufs=1, space="PSUM") as psA,
            tc.tile_pool(name="psD", bufs=1, space="PSUM") as psD,
            tc.tile_pool(name="psO", bufs=2, space="PSUM") as psO,
            tc.tile_pool(name="psX", bufs=2, space="PSUM") as psX,
        ):
            iota_t = cpool.tile([128, 128], bf16, tag="iota")
            nc.sync.dma_start(out=iota_t[:], in_=iota_in[:])
            ident_t = cpool.tile([128, 128], bf16, tag="ident")
            nc.sync.dma_start(out=ident_t[:], in_=ident_in[:])

            # resident per-graph constants + metadata
            w1l_t, w1r_t, w2l_t, w2r_t, b1_t, b1n_t, b2_t = [], [], [], [], [], [], []
            idx_t, dst_t = [], []
            for g in range(META):
                t = wpool.tile([F, H], bf16, tag=f"w1l{g}", name=f"w1l{g}")
                nc.sync.dma_start(out=t[:], in_=w1l_in[g])
                w1l_t.append(t)
                t = wpool.tile([F, H], bf16, tag=f"w1r{g}", name=f"w1r{g}")
                nc.sync.dma_start(out=t[:], in_=w1r_in[g])
                w1r_t.append(t)
                t = wpool.tile([H, D], bf16, tag=f"w2l{g}", name=f"w2l{g}")
                nc.sync.dma_start(out=t[:], in_=w2l_in[g])
                w2l_t.append(t)
                t = wpool.tile([H, D], bf16, tag=f"w2r{g}", name=f"w2r{g}")
                nc.sync.dma_start(out=t[:], in_=w2r_in[g])
                w2r_t.append(t)
                t = wpool.tile([H, 1], fp32, tag=f"b1{g}", name=f"b1{g}")
                nc.sync.dma_start(out=t[:], in_=b1_in[g])
                b1_t.append(t)
                t = wpool.tile([H, 1], fp32, tag=f"b1n{g}", name=f"b1n{g}")
                nc.sync.dma_start(out=t[:], in_=b1n_in[g])
                b1n_t.append(t)
                t = wpool.tile([128, D], fp32, tag=f"b2{g}", name=f"b2{g}")
                nc.sync.dma_start(out=t[:], in_=b2_in[g])
                b2_t.append(t)
                t = mpool.tile([128, totc[g] * 8], i16, tag=f"idx{g}", name=f"idx{g}")
                nc.sync.dma_start(
                    out=t[:], in_=idx_in[:, goff[g] * 8 : (goff[g] + totc[g]) * 8]
                )
                idx_t.append(t)
                t = mpool.tile([128, totc[g]], bf16, tag=f"dst{g}", name=f"dst{g}")
                nc.sync.dma_start(
                    out=t[:], in_=dst_in[:, goff[g] : goff[g] + totc[g]]
                )
                dst_t.append(t)

            hT = [
                hpool.tile([H, NSH], bf16, tag=f"hTg{g}", name=f"hTg{g}")
                for g in range(META)
            ]

            qrr = [0]

            def edge_phase(g, layer):
                """Gather + one-hot matmul accumulate for all groups of graph g.

                Both layers produce psum aggT [128feat x 128dst] per (block,
                seg) via lhsT=gathered bf16 rows, rhs=bf16 one-hot. Yields
                (b0, nb, psum_slice) per group once its psums are complete.
                """
                pool = psA if layer == 1 else psD
                for (gg, b0, nb, lo0, lon, hi0, hin) in groups:
                    if gg != g:
                        continue
                    # one psum region [128 cols] per (block, seg); 4 regions
                    # per bank; accumulation groups are sequential per region.
                    nbank = _ceil(nb * 2, 4)
                    ps = [
                        pool.tile(
                            [128, 512], fp32, tag=f"edge{layer}_{i}",
                            name=f"ps{layer}_{i}",
                        )
                        for i in range(nbank)
                    ]

                    def psum_slice(b, seg):
                        j = (b - b0) * 2 + seg
                        return ps[j // 4][:, (j % 4) * 128 : (j % 4) * 128 + 128]

                    started = set()
                    left = {}
                    for b in range(b0, b0 + nb):
                        left[(b, 0)] = int(pl[g, b])
                        left[(b, 1)] = int(ph[g, b])

                    for (c0, ncols) in ((lo0, lon), (hi0, hin)):
                        seg = 0 if c0 == lo0 else 1
                        if layer == 1:
                            src_ap = (
                                xbf_in[g, :, :] if seg == 0 else xbf_in[g, SPLIT:, :]
                            )
                        else:
                            src_ap = (
                                hfull[g][:, :] if seg == 0 else hfull[g][SPLIT:, :]
                            )
                        for w0 in range(c0, c0 + ncols, WIN):
                            wn = min(WIN, c0 + ncols - w0)
                            gt = gpool.tile([128, WIN, 128], bf16, tag="gt")
                            nc.gpsimd.dma_gather(
                                gt[:, :wn, :],
                                src_ap,
                                idx_t[g][:, w0 * 8 : (w0 + wn) * 8],
                                wn * 128,
                                wn * 128,
                                128,
                                queue_num=qrr[0] % NQUEUE,
                            )
                            qrr[0] += 1
                            oh = opool.tile([128, WIN, 128], bf16, tag="oh")
                            nc.vector.tensor_tensor(
                                out=oh[:, :wn, :],
                                in0=dst_t[g][:, w0 : w0 + wn]
                                .rearrange("p (w o) -> p w o", o=1)
                                .to_broadcast([128, wn, 128]),
                                in1=iota_t[:]
                                .rearrange("p (o d) -> p o d", o=1)
                                .to_broadcast([128, wn, 128]),
                                op=mybir.AluOpType.is_equal,
                            )
                            for j in range(wn):
                                b, sseg = stream[g][w0 + j]
                                first = (b, sseg) not in started
                                if first:
                                    started.add((b, sseg))
                                left[(b, sseg)] -= 1
                                nc.tensor.matmul(
                                    out=psum_slice(b, sseg),
                                    lhsT=gt[:, j, :],
                                    rhs=oh[:, j, :],
                                    start=first,
                                    stop=left[(b, sseg)] == 0,
                                    skip_group_check=True,
                                )
                    yield b0, nb, psum_slice

            # ================= layer 1 (+AllGather) per graph =================
            for g in range(META):
                for b0, nb, psl in edge_phase(g, 1):
                    nw = nb * 128
                    # per-group streamed xts/invb slices
                    xtg = dpool.tile([128, GRP * 128], bf16, tag="xtg")
                    nc.sync.dma_start(
                        out=xtg[:, :nw], in_=xts_in[g, :, b0 * 128 : b0 * 128 + nw]
                    )
                    ibg = dpool.tile([128, GRP * 128], bf16, tag="ibg")
                    nc.sync.dma_start(
                        out=ibg[:, :nw], in_=invb_in[g, :, b0 * 128 : b0 * 128 + nw]
                    )
                    # meanT per block -> m1grp bf16 [F, nb*128]
                    m1g = dpool.tile([F, GRP, 128], bf16, tag="m1g")
                    for bi in range(nb):
                        b = b0 + bi
                        mc = dpool.tile([F, 128], fp32, tag="mc")
                        nc.vector.tensor_copy(out=mc[:], in_=psl(b, 0))
                        s0 = dpool.tile([F, 128], bf16, tag="s0")
                        nc.vector.tensor_tensor(
                            out=s0[:], in0=psl(b, 1), in1=mc[:],
                            op=mybir.AluOpType.add,
                        )
                        nc.vector.tensor_tensor(
                            out=m1g[:, bi, :],
                            in0=s0[:],
                            in1=ibg[:, bi * 128 : bi * 128 + 128],
                            op=mybir.AluOpType.mult,
                        )
                    # dense: o1 = W1l^T @ mean1T + W1r^T @ xT  [H, nb*128]
                    o1 = psO.tile([H, 512], fp32, tag="o1", name="o1")
                    nc.tensor.matmul(
                        out=o1[:, :nw],
                        lhsT=w1l_t[g][:],
                        rhs=m1g[:, :nb, :],
                        start=True,
                        stop=False,
                    )
                    nc.tensor.matmul(
                        out=o1[:, :nw],
                        lhsT=w1r_t[g][:],
                        rhs=xtg[:, :nw],
                        start=False,
                        stop=True,
                    )
                    # ELU: h = relu(z) + exp(-relu(-z)) - 1, z = o1 + b1.
                    # relu/exp/identity(+bias) run on the scalar engine to
                    # keep DVE off the critical path.
                    tp = dpool.tile([H, 512], fp32, tag="tp")
                    nc.scalar.activation(
                        out=tp[:, :nw], in_=o1[:, :nw],
                        func=mybir.ActivationFunctionType.Relu,
                        bias=b1_t[g][:, :1],
                    )
                    tm = dpool.tile([H, 512], fp32, tag="tm")
                    nc.scalar.activation(
                        out=tm[:, :nw], in_=o1[:, :nw],
                        func=mybir.ActivationFunctionType.Relu,
                        bias=b1n_t[g][:, :1], scale=-1.0,
                    )
                    te = dpool.tile([H, 512], fp32, tag="te")
                    nc.scalar.activation(
                        out=te[:, :nw], in_=tm[:, :nw],
                        func=mybir.ActivationFunctionType.Exp, scale=-1.0,
                    )
                    ts_ = dpool.tile([H, 512], fp32, tag="ts")
                    nc.vector.tensor_tensor(
                        out=ts_[:, :nw], in0=te[:, :nw], in1=tp[:, :nw],
                        op=mybir.AluOpType.add,
                    )
                    hs = hT[g][:, b0 * 128 : b0 * 128 + nw]
                    nc.scalar.activation(
                        out=hs, in_=ts_[:, :nw],
                        func=mybir.ActivationFunctionType.Identity, bias=-1.0,
                    )
                    # h rows (bf16) via PE transpose per block; one DMA per grp
                    hr = qpool.tile([128, GRP, 128], bf16, tag="hr")
                    for bi in range(nb):
                        b = b0 + bi
                        htr = psX.tile([128, 128], bf16, tag="work", name="htr")
                        nc.tensor.transpose(
                            out=htr[:],
                            in_=hT[g][:, b * 128 : b * 128 + 128],
                            identity=ident_t[:],
                        )
                        nc.vector.tensor_copy(out=hr[:, bi, :], in_=htr[:])
                    nc.sync.dma_start(
                        out=hshard[g][b0 * 128 : b0 * 128 + nw, :].rearrange(
                            "(b p) d -> p b d", p=128
                        ),
                        in_=hr[:, :nb, :],
                    )

                # ---- allgather h rows ----
                nc.gpsimd.collective_compute(
                    "AllGather",
                    mybir.AluOpType.bypass,
                    ins=[hshard[g][:]],
                    outs=[hfull[g][:]],
                    replica_groups=[core_ids],
                )

            # ================= layer 2, all graphs =================
            for g in range(META):
                for b0, nb, psl in edge_phase(g, 2):
                    nw = nb * 128
                    ibg = dpool.tile([128, GRP * 128], bf16, tag="ibg2")
                    nc.sync.dma_start(
                        out=ibg[:, :nw], in_=invb_in[g, :, b0 * 128 : b0 * 128 + nw]
                    )
                    og = qpool.tile([128, GRP, D], fp32, tag="og")
                    for bi in range(nb):
                        b = b0 + bi
                        mc = dpool.tile([H, 128], fp32, tag="mc2")
                        nc.vector.tensor_copy(out=mc[:], in_=psl(b, 0))
                        s2 = dpool.tile([H, 128], bf16, tag="s2")
                        nc.vector.tensor_tensor(
                            out=s2[:], in0=psl(b, 1), in1=mc[:],
                            op=mybir.AluOpType.add,
                        )
                        m2 = dpool.tile([H, 128], bf16, tag="m2")
                        nc.vector.tensor_tensor(
                            out=m2[:],
                            in0=s2[:],
                            in1=ibg[:, bi * 128 : bi * 128 + 128],
                            op=mybir.AluOpType.mult,
                        )
                        # o2 = mean2 @ W2l + h @ W2r  [128dst, D]
                        o2 = psX.tile([128, D], fp32, tag="work", name="o2")
                        nc.tensor.matmul(
                            out=o2[:], lhsT=m2[:], rhs=w2l_t[g][:],
                            start=True, stop=False,
                        )
                        nc.tensor.matmul(
                            out=o2[:],
                            lhsT=hT[g][:, b * 128 : b * 128 + 128],
                            rhs=w2r_t[g][:],
                            start=False,
                            stop=True,
                        )
                        nc.vector.tensor_tensor(
                            out=og[:, bi, :], in0=o2[:], in1=b2_t[g][:],
                            op=mybir.AluOpType.add,
                        )
                    # batched log_softmax along D for the whole group
                    rmax = dpool.tile([128, GRP, 1], fp32, tag="rmax")
                    nc.vector.reduce_max(
                        out=rmax[:, :nb, :], in_=og[:, :nb, :],
                        axis=mybir.AxisListType.X,
                    )
                    x1 = qpool.tile([128, GRP, D], fp32, tag="x1")
                    nc.vector.tensor_tensor(
                        out=x1[:, :nb, :],
                        in0=og[:, :nb, :],
                        in1=rmax[:, :nb, :].to_broadcast([128, nb, D]),
                        op=mybir.AluOpType.subtract,
                    )
                    ex = qpool.tile([128, GRP, D], fp32, tag="ex")
                    nc.scalar.activation(
                        out=ex[:, :nb, :], in_=x1[:, :nb, :],
                        func=mybir.ActivationFunctionType.Exp,
                    )
                    sm = dpool.tile([128, GRP, 1], fp32, tag="sm")
                    nc.vector.reduce_sum(
                        out=sm[:, :nb, :], in_=ex[:, :nb, :],
                        axis=mybir.AxisListType.X,
                    )
                    ls = dpool.tile([128, GRP, 1], fp32, tag="ls")
                    nc.scalar.activation(
                        out=ls[:, :nb, :], in_=sm[:, :nb, :],
                        func=mybir.ActivationFunctionType.Ln,
                    )
                    ob = qpool.tile([128, GRP, D], fp32, tag="ob")
                    nc.vector.tensor_tensor(
                        out=ob[:, :nb, :],
                        in0=x1[:, :nb, :],
                        in1=ls[:, :nb, :].to_broadcast([128, nb, D]),
                        op=mybir.AluOpType.subtract,
                    )
                    nc.sync.dma_start(
                        out=out_ext[g, b0 * 128 : b0 * 128 + nw, :].rearrange(
                            "(b p) d -> p b d", p=128
                        ),
                        in_=ob[:, :nb, :],
                    )

    nc.finalize()
    return nc


def kernel(**inputs):
    out, _ = run_kernel(inputs)
    return out


def run_kernel(inputs, trace=False):
    from concourse.bass_utils import run_bass_kernel_spmd

    meta_x = np.asarray(inputs["meta_x"], dtype=np.float32)
    layout, per_core = _prep_host(meta_x, inputs["meta_edge_index"])
    nc = _build_program(layout)

    xbf = meta_x.astype(BF16)
    iota = np.tile(np.arange(128, dtype=np.float32), (128, 1)).astype(BF16)
    ident = np.eye(128, dtype=np.float32).astype(BF16)
    w1l = np.asarray(inputs["W1l"], dtype=np.float32).astype(BF16)
    w1r = np.asarray(inputs["W1r"], dtype=np.float32).astype(BF16)
    w2l = np.asarray(inputs["W2l"], dtype=np.float32).astype(BF16)
    w2r = np.asarray(inputs["W2r"], dtype=np.float32).astype(BF16)
    b1 = np.asarray(inputs["b1"], dtype=np.float32)
    b2 = np.asarray(inputs["b2"], dtype=np.float32)
    b1c = b1[:, :, None].copy()
    b1nc = -b1c
    b2b = np.broadcast_to(b2[:, None, :], (META, 128, D)).copy()

    in_maps = []
    for c in range(NCORES):
        pc = per_core[c]
        in_maps.append(
            {
                "xbf": xbf,
                "idx": pc["idxw"],
                "dstw": pc["dstw"],
                "xts": pc["xts"],
                "invb": pc["invb"],
                "w1l": w1l,
                "w1r": w1r,
                "b1c": b1c,
                "b1n": b1nc,
                "w2l": w2l,
                "w2r": w2r,
                "b2b": b2b,
                "iota": iota,
                "ident": ident,
            }
        )

    res = run_bass_kernel_spmd(nc, in_maps, list(range(NCORES)), trace=trace)
    out = np.zeros((META, N, D), dtype=np.float32)
    for c in range(NCORES):
        lo = c * NSH
        n = min(NSH, N - lo)
        out[:, lo : lo + n, :] = res.results[c]["out"][:, :n, :]
    return out, res
